# revision 2
# baseline (speedup 1.0000x reference)
"""Trainium2 Bass kernel v2 for nn_DisLoss: loss = sum(x * dist_to_argmax(x)) / b.

x: (128, 512, 512) f32. Data-parallel over 8 NeuronCores: 16 images per core.

Same distance-field factorization as v1 (rank-16 eigendecomposition of the
1024x1024 distance table; U-side contraction on PE via a cy-shifted gather of
W rows; V-side + exact cx on the host in float64). The device-side argmax
chain is BATCHED per group of 4 images to collapse the per-pair serial
latency that dominated v1:

  per group g (images 4g..4g+3):
    s0  4x 1MB HWDGE DMA + 4x DVE rowmax scan -> rowmax_g [128, im, t]
    s1  ONE gpsimd partition_all_reduce(max) [128, 16] -> msb2 (colmax per
        (im,t) broadcast to all partitions)
    s2  DVE: pm2 = reduce_t(msb2) [128,4] (global max per image, everywhere)
        eq = (rowmax == pm2_bcast); ep = reduce_t(eq * ENC) [128,4]
    s3  ONE allreduce(max) -> e1b [128,4] = encoded argmax row, everywhere
    s4  woff/cyu offsets (2 DVE stt) + ONE multi-column W-gather
        [128,4]->[128,4,64] + ONE row-gather [128,4]->[128,4,16]
    s5  per image: 4 fp32r matmuls -> y [16,512] PSUM
    s6  per image: ACT copy PSUM->SBUF; per group: ys DMA (ACT-ring)

x DMAs ride the SP HWDGE ring; output DMAs ride the ACT ring so they never
block the x stream. Groups are software-pipelined 2 deep (chain of group g
runs under the DMA stream of group g+1).
"""

import numpy as np

B_FULL = 128
H = 512
W = 512
N_CORES = 8
B_CORE = B_FULL // N_CORES  # 16 images per core
T = 4                       # rows per partition
K = 16                      # factor rank
G = 4                       # max images per group
GROUPS = [4, 4, 4, 4]       # group sizes (sum = B_CORE)
NG = len(GROUPS)
GSTART = [sum(GROUPS[:i]) for i in range(NG)]
ROWSEG = 16                 # elems per row segment in the row gather

_CACHE = {}


def _factors():
    i = np.arange(1024, dtype=np.float64) - 512.0
    Tm = np.sqrt(i[:, None] ** 2 + i[None, :] ** 2)
    lam, Q = np.linalg.eigh(Tm)
    idx = np.argsort(-np.abs(lam))[:K]
    lam_k = lam[idx]
    Wh = Q[:, idx] * np.sqrt(np.abs(lam_k))[None, :]   # [1024, K] float64
    sigma = np.sign(lam_k)                              # [K]
    return Wh, sigma


def _consts():
    Wh, _sigma = _factors()
    Wf = Wh.astype(np.float32)                          # [1024, K]
    wq = np.zeros((1024, 4 * K), dtype=np.float32)
    for a in range(4):
        hi = 1024 - a
        wq[:hi, a * K:(a + 1) * K] = Wf[a:, :]
    p = np.arange(128, dtype=np.float32)[:, None]
    t = np.arange(T, dtype=np.float32)[None, :]
    r4enc = (32.0 * (4096.0 - (4.0 * p + t))).copy()    # [128, T]
    pidxm = np.tile(4.0 * p - 3584.0, (1, G)).copy()    # [128, G]
    b = np.arange(B_CORE, dtype=np.float32)[None, :]
    pidxc = (b * 16384.0 + 131072.0 + (p % 32)).copy()  # [128, 16]
    return {"WQUAD": wq, "R4ENC": r4enc, "PIDXM": pidxm, "PIDXC": pidxc}


def build_program(debug=False, hwloop=0, stages_upto=None, xbufs=4,
                  pair_dma=True, groups=None, bcast_eq=False,
                  gp_offsets=False):
    import concourse.bass as bass
    import concourse.bass_isa as bass_isa
    import concourse.bacc as bacc
    import concourse.mybir as mybir
    from concourse.tile import TileContext

    nb = B_CORE
    f32 = mybir.dt.float32
    f32r = mybir.dt.float32r
    u32 = mybir.dt.uint32
    Alu = mybir.AluOpType
    Ax = mybir.AxisListType

    su = 99 if stages_upto is None else stages_upto
    grp = GROUPS if groups is None else groups
    ng = len(grp)
    gstart = [sum(grp[:i]) for i in range(ng)]

    nc = bacc.Bacc("TRN2", target_bir_lowering=False, debug=False)

    x_d = nc.dram_tensor("x", [nb, H, W], f32r, kind="ExternalInput")
    wq_d = nc.dram_tensor("WQUAD", [1024, 4 * K], f32r, kind="ExternalInput")
    r4enc_d = nc.dram_tensor("R4ENC", [128, T], f32, kind="ExternalInput")
    pidxm_d = nc.dram_tensor("PIDXM", [128, G], f32, kind="ExternalInput")
    pidxc_d = nc.dram_tensor("PIDXC", [128, nb], f32, kind="ExternalInput")
    ys_d = nc.dram_tensor("ys", [K, nb * W], f32, kind="ExternalOutput")
    e1b_d = nc.dram_tensor("e1bout", [1, nb], f32, kind="ExternalOutput")
    rows_d = nc.dram_tensor("rowsout", [128, ng * ROWSEG], f32,
                            kind="ExternalOutput")

    x_ap = x_d.ap()
    x_rows = x_ap.rearrange("b h (s i) -> (b h s) i", i=ROWSEG)
    wq_rows = wq_d.ap()

    with TileContext(nc) as tc:
        with (
            tc.tile_pool(name="consts", bufs=1) as consts,
            tc.tile_pool(name="xs", bufs=xbufs) as xs,
            tc.tile_pool(name="small", bufs=3) as small,
            tc.tile_pool(name="psy", bufs=7, space="PSUM") as psy,
        ):
            r4enc_t = consts.tile([128, T], f32)
            nc.sync.dma_start(out=r4enc_t, in_=r4enc_d.ap())
            pidxm_t = consts.tile([128, G], f32)
            nc.sync.dma_start(out=pidxm_t, in_=pidxm_d.ap())
            pidxc_t = consts.tile([128, nb], f32)
            nc.sync.dma_start(out=pidxc_t, in_=pidxc_d.ap())

            ys_all = consts.tile([K, nb * W], f32)
            e1b_all = consts.tile([128, nb], f32)
            rows_all = consts.tile([128, ng * ROWSEG], f32)
            # sz<4 groups leave the upper partitions of their rows_all
            # column unwritten; zero once so the end-DMA reads defined data
            nc.vector.memset(rows_all, 0.0)
            if stages_upto is not None:
                nc.vector.memset(ys_all, 0.0)
                nc.vector.memset(e1b_all, 0.0)

            def s0(st):
                b0, sz = st["b0"], st["sz"]
                x_t = xs.tile([128, sz, T, W], f32r, tag="x")
                rowmax = small.tile([128, sz, T], f32, tag="rm")
                if pair_dma and st["g"] < ng - 1:
                    for im in range(0, sz, 2):
                        nc.sync.dma_start(
                            out=x_t[:, im:im + 2, :, :],
                            in_=x_ap[b0 + im:b0 + im + 2].rearrange(
                                "b (p t) w -> p b t w", p=128))
                else:
                    for im in range(sz):
                        nc.sync.dma_start(
                            out=x_t[:, im, :, :],
                            in_=x_ap[b0 + im].rearrange(
                                "(p t) w -> p t w", p=128))
                st.update(x_t=x_t, rowmax=rowmax)

            def s0scan(st):
                rowmax = st["rowmax"]
                for im in range(st["sz"]):
                    nc.vector.tensor_reduce(
                        rowmax[:, im, :],
                        st["x_t"][:, im, :, :].bitcast(f32),
                        axis=Ax.X, op=Alu.max)

            def chain(st):
                g, b0, sz = st["g"], st["b0"], st["sz"]
                rowmax = st["rowmax"]
                msb2 = small.tile([128, sz, T], f32, tag="msb2")
                if su >= 2:
                    nc.gpsimd.partition_all_reduce(
                        msb2.rearrange("p a b -> p (a b)"),
                        rowmax.rearrange("p a b -> p (a b)"),
                        channels=128, reduce_op=bass_isa.ReduceOp.max)
                if su >= 3:
                    pm2 = small.tile([128, sz], f32, tag="pm2")
                    nc.vector.tensor_reduce(pm2, msb2, axis=Ax.X, op=Alu.max)
                    ep = small.tile([128, sz], f32, tag="ep")
                    if bcast_eq:
                        eqt = small.tile([128, sz, T], f32, tag="eqt")
                        nc.vector.tensor_tensor(
                            eqt, rowmax,
                            pm2.rearrange("p (a o) -> p a o", o=1)
                            .broadcast_to([128, sz, T]), op=Alu.is_equal)
                        nc.vector.tensor_tensor(
                            eqt, eqt,
                            r4enc_t.rearrange("p (o b) -> p o b", o=1)
                            .broadcast_to([128, sz, T]), op=Alu.mult)
                        nc.vector.tensor_reduce(ep, eqt, axis=Ax.X,
                                                op=Alu.max)
                    else:
                        eqj = small.tile([128, T], f32, tag="eqj")
                        for im in range(sz):
                            # (rowmax == global-max) * enc, summed over t:
                            # the unique fp32-exact match makes the sum the
                            # encoding
                            nc.vector.scalar_tensor_tensor(
                                eqj, rowmax[:, im, :], pm2[:, im:im + 1],
                                r4enc_t, op0=Alu.is_equal, op1=Alu.mult,
                                accum_out=ep[:, im:im + 1])
                e1b = e1b_all[:, b0:b0 + sz]
                if su >= 4:
                    nc.gpsimd.partition_all_reduce(
                        e1b, ep, channels=128,
                        reduce_op=bass_isa.ReduceOp.max)
                if su >= 5:
                    # single-column offset APs only: the multi-column
                    # indirect-DMA form returns garbage on real SWDGE
                    # (CoreSim accepts it; hardware does not)
                    off_eng = nc.gpsimd if gp_offsets else nc.vector
                    wqt = small.tile([128, sz, 4 * K], f32r, tag="wq")
                    for im in range(sz):
                        woff = small.tile([128, 1], u32, tag=f"woff{im}")
                        off_eng.scalar_tensor_tensor(
                            woff, e1b[:, im:im + 1], 1.0 / 32.0,
                            pidxm_t[:, 0:1], op0=Alu.mult, op1=Alu.add)
                        nc.gpsimd.indirect_dma_start(
                            out=wqt[:, im, :], out_offset=None,
                            in_=wq_rows,
                            in_offset=bass.IndirectOffsetOnAxis(
                                ap=woff[:], axis=0))
                    cyu = small.tile([128, 1], u32, tag="cyu")
                    for a in range(sz):
                        sl = slice(32 * a, 32 * (a + 1))
                        off_eng.scalar_tensor_tensor(
                            cyu[sl, 0:1], e1b[sl, a:a + 1], -1.0,
                            pidxc_t[sl, b0 + a:b0 + a + 1],
                            op0=Alu.mult, op1=Alu.add)
                    nc.gpsimd.indirect_dma_start(
                        out=rows_all[0:32 * sz, ROWSEG * g:
                                     ROWSEG * (g + 1)].bitcast(f32r),
                        out_offset=None,
                        in_=x_rows,
                        in_offset=bass.IndirectOffsetOnAxis(
                            ap=cyu[0:32 * sz, :], axis=0))
                    st.update(wqt=wqt)

            def mm(st):
                b0, sz = st["b0"], st["sz"]
                x_t, wqt = st["x_t"], st["wqt"]
                for im in range(sz):
                    b = b0 + im
                    y_ps = psy.tile([K, W], f32, tag="y")
                    for t in range(T):
                        nc.tensor.matmul(
                            y_ps, wqt[:, im, t * K:(t + 1) * K],
                            x_t[:, im, t, :],
                            start=(t == 0), stop=(t == T - 1))
                    if su >= 7:
                        nc.scalar.copy(
                            ys_all[:, b * W:(b + 1) * W], y_ps)
                if su >= 7:
                    nc.scalar.dma_start(
                        out=ys_d.ap()[:, b0 * W:(b0 + sz) * W],
                        in_=ys_all[:, b0 * W:(b0 + sz) * W])

            def body():
                states = {}
                for i in range(ng + 1):
                    if i < ng:
                        states[i] = {"g": i, "b0": gstart[i],
                                     "sz": grp[i]}
                        s0(states[i])
                    if i - 1 >= 0:
                        chain(states[i - 1])
                        if i - 1 == ng - 1 and su >= 7:
                            # small end-DMAs ride the ACT ring ahead of the
                            # last group's ys so nothing trails the matmuls
                            nc.scalar.dma_start(out=e1b_d.ap(),
                                                in_=e1b_all[0:1, :])
                            nc.scalar.dma_start(out=rows_d.ap(),
                                                in_=rows_all)
                    if i < ng:
                        s0scan(states[i])
                    if i - 1 >= 0 and su >= 6:
                        mm(states.pop(i - 1))

            if hwloop:
                with tc.For_i(0, hwloop):
                    body()
            else:
                body()

    nc.compile()
    return nc


def _host_reduce(results):
    """Apply the V-side contraction in float64 on the host."""
    Wh, sigma = _factors()
    Ws = Wh * sigma[None, :]                      # [1024, K] float64
    total = 0.0
    cidx = np.arange(W)
    for r in results:
        ys = r["ys"].astype(np.float64)           # [K, nb*W]
        e1b = r["e1bout"][0]                      # [nb]
        rowsout = r["rowsout"]                    # [128, NG*ROWSEG]
        for b in range(B_CORE):
            g = max(i for i in range(NG) if GSTART[i] <= b)
            a = b - GSTART[g]
            row = rowsout[32 * a:32 * (a + 1),
                          g * ROWSEG:(g + 1) * ROWSEG].reshape(-1)
            cx = int(np.argmax(row))
            y = ys[:, b * W:(b + 1) * W]          # [K, 512]
            Vs = Ws[cidx - cx + 512, :]           # [512, K]
            total += float(np.einsum("kc,ck->", y, Vs))
    return total


def kernel(x: np.ndarray) -> np.ndarray:
    from concourse import bass_utils

    key = "nc"
    if key not in _CACHE:
        _CACHE[key] = build_program(debug=False)
    nc = _CACHE[key]

    x = np.ascontiguousarray(x, dtype=np.float32)
    shards = x.reshape(N_CORES, B_CORE, H, W)
    consts = _consts()
    in_maps = [dict(consts, x=shards[i]) for i in range(N_CORES)]
    res = bass_utils.run_bass_kernel_spmd(
        nc, in_maps, core_ids=list(range(N_CORES)))
    total = _host_reduce(res.results)
    return np.float32(total / B_FULL)
